# revision 10
# baseline (speedup 1.0000x reference)
"""BertSelfAttention with gated prompt-prefix branch on 8 Trainium2 cores.

Sharding: data-parallel over batch (B=8 -> 1 batch element per core), no
collectives. Per core, the full attention pipeline runs in a transposed
[feature, seq] layout so that softmax statistics ride through the matmuls:

  qT/kT = W @ hsT          [768, 1024]  (bf16, PE)
  v_aug = hs @ WvT_aug     [1024, 780]  natural layout, 65-col stride per
                           head, col 65h+64 = ones (denominator column)
  scoresT_h = kh @ qh.T    [t, s] via K=64 row-tiled matmuls, 2 heads
                           concurrently on the 128x128 PE array
  expT = exp(SCALE*scoresT)  split between the ACT engine (true exp) and
         the DVE (Schraudolph bit-trick: bf16_bits = int16(x*a + b)) so
         that neither drain engine ever paces the PE.  The PE p-state
         ramp resets whenever a PE instruction blocks on an unsatisfied
         semaphore, halving matmul throughput; the whole schedule is
         built to keep every PE wait pre-satisfied.
  ctxT_aug_h = v_aug_h.T @ expT_h       rows 0..63 ctx, row 64 = sum_t exp
  ctx matmuls for pair c-1 interleave with scores for pair c (a full
  pair of lag), and the Q/K/V/prompt projections are spread through the
  pair-0 score phase as PE filler.
  out_h = ctxT/denom + pctxT/pdenom     (DVE mul from PSUM + GpSimd
          mul/add; reciprocal + partition-broadcast ride the DMA rings)

Output is produced as outT [768, 1024] fp32 per core; the host transposes
and stacks to [8, 1024, 768].
"""

import numpy as np
import ml_dtypes

import concourse.bass as bass
import concourse.mybir as mybir
import concourse.tile as tile
from concourse.bass_utils import run_bass_kernel_spmd
from concourse.vector_clock import ScopedClock


class SplitDrainTileContext(tile.TileContext):
    """This walrus build rejects >2 sync waits on the kernel-tail Drain
    ("Too many sync wait commands"); split them across SP nops instead."""

    def _drain_and_barrier(self, tick_clock, wait_clock):
        probe = self.nc.sync.nop(nofuse=True, hint="drain_wait_split")
        wait_clock.add_sem_waits(
            probe.ins, ScopedClock({None: tick_clock.global_clock})
        )
        waits = list(probe.ins.sync_info.on_wait or [])
        if len(waits) > 1:
            probe.ins.sync_info.on_wait = waits[:1]
            for i in range(1, len(waits)):
                extra = self.nc.sync.nop(nofuse=True, hint="drain_wait_split")
                extra.ins.sync_info = mybir.SyncInfo(
                    on_wait=waits[i : i + 1], on_update=[]
                )
        drain_inst = self.nc.sync.drain()
        if drain_inst.ins.sync_info is not None:
            drain_inst.ins.sync_info.on_wait = []
        self.nc.all_engine_barrier()
        assert self.sems is not None
        popped = self.nc._tile_sem_poison_stack.pop()
        assert popped is self._sem_poison
        self.nc.clear_and_free_semaphores(list(self.sems.allocated().values()))
        self.nc.all_engine_barrier()

F32 = mybir.dt.float32
BF16 = mybir.dt.bfloat16
I16 = mybir.dt.int16
AF = mybir.ActivationFunctionType
ALU = mybir.AluOpType

H, DH, D = 12, 64, 768
S, AT, B = 1024, 64, 8
SCALE = 1.0 / np.sqrt(DH)
NC_D = D // 128  # 6 chunks over feature dim
NC_S = S // 128  # 8 chunks over sequence dim
PAIRS = H // 2  # 6 head pairs
VW = H * (DH + 1)  # 780: v with per-head ones column

# Schraudolph exp -> bf16 bits: bits = trunc(x * EXP_A + EXP_B), bitcast
# int16 -> bf16 gives ~exp(SCALE*x) with ~1.8% rms error that averages
# out inside the softmax-weighted context sum.
EXP_A = float(SCALE * 128.0 / np.log(2.0))
EXP_B = 16256.0 - 6.75

_CACHE = {}
LAST_RESULTS = None


_DVE_TILES = {(1, 1), (2, 1), (3, 1), (4, 1), (5, 1), (3, 0), (4, 0)}


def _exp_engine(tci, half):
    # DVE takes 7 of the 16 main exp tiles per pair; ACT the rest + prefix.
    # Keeping tci 6-7 on ACT leaves the DVE free for the end-of-block
    # normalization ops.
    return "dve" if (tci, half) in _DVE_TILES else "act"


def _split_sync_waits(nc, cap=1):
    """Walrus on this image allows very few sync-wait commands per
    instruction (tensor_scalar rejects 2). Hoist excess waits onto
    same-engine nops placed immediately before the instruction."""
    for bb in nc.main_func.blocks:
        cur = list(bb.instructions)
        out = []
        for inst in cur:
            si = inst.sync_info
            waits = list(si.on_wait) if si and si.on_wait else []
            if len(waits) > cap:
                for i in range(0, len(waits) - cap):
                    bi = nc.engines[inst.engine].nop(
                        nofuse=True, hint="wait_split")
                    popped = nc.cur_bb.bb.instructions.pop()
                    assert popped is bi.ins
                    bi.ins.sync_info = mybir.SyncInfo(
                        on_wait=waits[i : i + 1], on_update=[])
                    out.append(bi.ins)
                si.on_wait = waits[len(waits) - cap:]
            out.append(inst)
        bb.instructions[:] = out


def _build_nc():
    nc = bass.Bass()
    hsT = nc.dram_tensor("hsT", [D, S], BF16, kind="ExternalInput")
    wqT = nc.dram_tensor("wqT", [D, D], BF16, kind="ExternalInput")
    wkT = nc.dram_tensor("wkT", [D, D], BF16, kind="ExternalInput")
    wvT = nc.dram_tensor("wvT", [D, VW], BF16, kind="ExternalInput")
    bq = nc.dram_tensor("bq", [D, 1], F32, kind="ExternalInput")
    bk = nc.dram_tensor("bk", [D, 1], F32, kind="ExternalInput")
    bvaug = nc.dram_tensor("bvaug", [128, VW], F32, kind="ExternalInput")
    promptT = nc.dram_tensor("promptT", [D, AT], BF16, kind="ExternalInput")
    mask = nc.dram_tensor("mask", [S, 1], F32, kind="ExternalInput")
    gating = nc.dram_tensor("gating", [128, VW], F32, kind="ExternalInput")
    outT = nc.dram_tensor("outT", [D, S], F32, kind="ExternalOutput")

    with SplitDrainTileContext(nc) as tc:
        _emit(nc, tc, hsT, wqT, wkT, wvT, bq, bk, bvaug, promptT, mask,
              gating, outT)
    _split_sync_waits(nc)
    return nc


def _emit(nc, tc, hsT, wqT, wkT, wvT, bq, bk, bvaug, promptT, mask, gating,
          outT):
    from contextlib import ExitStack

    with ExitStack() as ctx:
        pers = ctx.enter_context(tc.tile_pool(name="pers", bufs=1))

        # ---- SBUF arrays that live into the attention phase ----
        mask_sb = pers.tile([128, NC_S], F32, tag="mask")
        emask_sb = pers.tile([128, NC_S], F32, tag="emask")
        qT_sb = pers.tile([128, NC_D * S], BF16, tag="qT")
        kT_sb = pers.tile([128, NC_D * S], BF16, tag="kT")
        v_sb = pers.tile([128, NC_S * VW], BF16, tag="v")
        pkT_sb = pers.tile([128, NC_D * AT], BF16, tag="pkT")
        pv_sb = pers.tile([128, VW], BF16, tag="pv")

        # ---- projection-phase-only arrays (pool closed afterwards so the
        # attention pools can reuse the space) ----
        proj_cm = tc.tile_pool(name="proj", bufs=1, side="right")
        proj = proj_cm.__enter__()
        hsT_sb = proj.tile([128, NC_D * S], BF16, tag="hsT")
        wqT_sb = proj.tile([128, NC_D * D], BF16, tag="wqT")
        wkT_sb = proj.tile([128, NC_D * D], BF16, tag="wkT")
        wvT_sb = proj.tile([128, NC_D * VW], BF16, tag="wvT")
        pT_sb = proj.tile([128, NC_D * AT], BF16, tag="pT")
        bq_sb = proj.tile([128, NC_D], F32, tag="bq")
        bk_sb = proj.tile([128, NC_D], F32, tag="bk")
        bvaug_sb = proj.tile([128, VW], F32, tag="bvaug")
        graw_sb = proj.tile([128, VW], F32, tag="graw")
        gbc_sb = proj.tile([128, VW], F32, tag="gbc")
        pvtmp_sb = proj.tile([64, VW], F32, tag="pvtmp")

        for src, dst, w in ((wqT, wqT_sb, D), (hsT, hsT_sb, S),
                            (wkT, wkT_sb, D), (wvT, wvT_sb, VW),
                            (promptT, pT_sb, AT)):
            nc.sync.dma_start(
                dst[:].rearrange("p (c s) -> p c s", s=w),
                src[:, :].rearrange("(c p) s -> p c s", p=128))
        # biases / mask: [768,1] & [1024,1] -> [128, nchunks]
        nc.sync.dma_start(bq_sb[:], bq.rearrange("(c p) 1 -> p c", p=128))
        nc.sync.dma_start(bk_sb[:], bk.rearrange("(c p) 1 -> p c", p=128))
        nc.sync.dma_start(mask_sb[:], mask.rearrange("(c p) 1 -> p c", p=128))
        nc.sync.dma_start(bvaug_sb[:], bvaug[:])
        # gating arrives host-replicated to [128, 780] (65 copies per head
        # along the row, broadcast down the partitions)
        nc.sync.dma_start(graw_sb[:], gating[:])
        # tanh, then force the ones-column slots back to 1.0
        nc.scalar.activation(gbc_sb[:], graw_sb[:], AF.Tanh)
        ones_slots = gbc_sb[:, :].rearrange(
            "p (h e) -> p h e", h=H)[:, :, DH:DH + 1]
        nc.vector.memset(ones_slots, 1.0)
        # e^mask, folded into the V rows (incl. ones column) instead of an
        # exp bias: exp(S*x + m_t) == e^{m_t} * exp(S*x), and the ones
        # column then accumulates the correctly-masked denominator.
        nc.scalar.activation(emask_sb[:], mask_sb[:], AF.Exp)

        # SBUF pools that outlive the projection phase — opened before the
        # closeable PSUM pools so the per-side pool stack unwinds LIFO
        exp_pool = ctx.enter_context(tc.tile_pool(name="expp", bufs=4))
        pexp_pool = ctx.enter_context(tc.tile_pool(name="pexpp", bufs=3))

        # ---- PSUM pools for the projection phase (closed afterwards) ----
        mm_cm = tc.tile_pool(name="mm", bufs=2, space="PSUM")
        mm_pool = mm_cm.__enter__()
        sc0_cm = tc.tile_pool(name="scp0", bufs=2, space="PSUM")
        scp = {"p": sc0_cm.__enter__()}

        def emit_exp(dst, st, engine, sb):
            """Exp one 512-col half of a scores tile: draining starts right
            after the first matmul, which keeps the psum-rotation wait
            pre-satisfied when the PE sequencer reaches it."""
            lo, hi = sb * 512, (sb + 1) * 512
            if engine == "act":
                nc.scalar.activation(dst[:, lo:hi], st[:, lo:hi], AF.Exp,
                                     scale=SCALE)
            else:
                nc.vector.tensor_scalar(
                    dst[:, lo:hi].bitcast(I16), st[:, lo:hi], EXP_A, EXP_B,
                    op0=ALU.mult, op1=ALU.add)

        def scores_tci(c, tci, exp_ab):
            """Scores + exp for (pair c, key-chunk tci), 2 heads row-tiled."""
            for half in range(2):
                hp = half * 64
                st = scp["p"].tile([128, S], F32, tag="sc",
                                   name=f"st_{c}_{tci}_{half}")
                lhsT = kT_sb[hp:hp + 64,
                             c * S + tci * 128: c * S + (tci + 1) * 128]
                eng = _exp_engine(tci, half)
                dst = exp_ab[half][:, tci * S:(tci + 1) * S]
                for sb in range(2):
                    nc.tensor.matmul(
                        st[:, sb * 512:(sb + 1) * 512], lhsT,
                        qT_sb[hp:hp + 64,
                              c * S + sb * 512: c * S + (sb + 1) * 512],
                        tile_position=(hp, 0))
                    emit_exp(dst, st, eng, sb)

        def prefix_scores(c, pexp):
            psp = scp["p"].tile([128, S], F32, tag="sc", name=f"psp_{c}")
            for half in range(2):
                hp = half * 64
                for sb in range(2):
                    nc.tensor.matmul(
                        psp[hp:hp + 64, sb * 512:(sb + 1) * 512],
                        pkT_sb[hp:hp + 64, c * AT:(c + 1) * AT],
                        qT_sb[hp:hp + 64,
                              c * S + sb * 512: c * S + (sb + 1) * 512],
                        tile_position=(hp, hp))
            nc.scalar.activation(pexp[:], psp[:], AF.Exp, scale=SCALE)

        # ---- projection helpers (run as PE filler between score chunks) ----
        def qk_chain(c, w_sb, b_sb, o_sb):
            ps = mm_pool.tile([128, S], F32, tag="mm")
            for kc in range(NC_D):
                lhsT = w_sb[:, kc * D + c * 128: kc * D + (c + 1) * 128]
                for sb in range(2):
                    nc.tensor.matmul(
                        ps[:, sb * 512:(sb + 1) * 512], lhsT,
                        hsT_sb[:, kc * S + sb * 512: kc * S + (sb + 1) * 512],
                        start=(kc == 0), stop=(kc == NC_D - 1))
            nc.vector.tensor_scalar_add(o_sb[:, c * S:(c + 1) * S],
                                        ps[:], b_sb[:, c:c + 1])

        def v_chunk(sc):
            ps = mm_pool.tile([128, S], F32, tag="mm")
            for kc in range(NC_D):
                lhsT = hsT_sb[:, kc * S + sc * 128: kc * S + (sc + 1) * 128]
                nc.tensor.matmul(ps[:, 0:512], lhsT,
                                 wvT_sb[:, kc * VW: kc * VW + 512],
                                 start=(kc == 0), stop=(kc == NC_D - 1))
                nc.tensor.matmul(ps[:, 512:VW], lhsT,
                                 wvT_sb[:, kc * VW + 512: (kc + 1) * VW],
                                 start=(kc == 0), stop=(kc == NC_D - 1))
            vt = proj.tile([128, VW], F32, tag="vtmp", name=f"vt{sc}",
                           bufs=2)
            nc.vector.tensor_add(vt[:], ps[:, 0:VW], bvaug_sb[:])
            nc.vector.tensor_scalar_mul(v_sb[:, sc * VW:(sc + 1) * VW],
                                        vt[:], emask_sb[:, sc:sc + 1])

        # ---- projection phase: pair-0 scores ride between filler chains ----
        qk_chain(0, wqT_sb, bq_sb, qT_sb)
        qk_chain(0, wkT_sb, bk_sb, kT_sb)
        fillers = []
        for c in range(1, NC_D):
            fillers.append(lambda c=c: qk_chain(c, wqT_sb, bq_sb, qT_sb))
            fillers.append(lambda c=c: qk_chain(c, wkT_sb, bk_sb, kT_sb))
        exps = {0: [exp_pool.tile([128, NC_S * S], BF16, tag="exp",
                                  name=f"exp_0_{i}") for i in range(2)]}
        fi = 0
        for tci in range(NC_S):
            scores_tci(0, tci, exps[0])
            n = 2 if tci < 2 else 1
            for _ in range(n):
                if fi < len(fillers):
                    fillers[fi]()
                    fi += 1
            v_chunk(tci)
        while fi < len(fillers):
            fillers[fi]()
            fi += 1

        # ---- prompt K projection (transposed) ----
        for c in range(NC_D):
            ps = mm_pool.tile([128, S], F32, tag="mm")
            for kc in range(NC_D):
                nc.tensor.matmul(
                    ps[:, 0:AT],
                    wkT_sb[:, kc * D + c * 128: kc * D + (c + 1) * 128],
                    pT_sb[:, kc * AT:(kc + 1) * AT],
                    start=(kc == 0), stop=(kc == NC_D - 1))
            nc.vector.tensor_scalar_add(pkT_sb[:, c * AT:(c + 1) * AT],
                                        ps[:, 0:AT], bk_sb[:, c:c + 1])

        # pair-0 prefix scores before the prompt-V chain so pexp(0) lands
        # well before the first attention block consumes the banks
        pexps = {0: pexp_pool.tile([128, S], BF16, tag="pexp", name="pexp0")}
        prefix_scores(0, pexps[0])

        # ---- prompt V projection (natural, gate-scaled, duplicated) ----
        ps = mm_pool.tile([128, S], F32, tag="mm")
        for kc in range(NC_D):
            lhsT = pT_sb[:, kc * AT:(kc + 1) * AT]
            nc.tensor.matmul(ps[0:AT, 0:512], lhsT,
                             wvT_sb[:, kc * VW: kc * VW + 512],
                             start=(kc == 0), stop=(kc == NC_D - 1))
            nc.tensor.matmul(ps[0:AT, 512:VW], lhsT,
                             wvT_sb[:, kc * VW + 512: (kc + 1) * VW],
                             start=(kc == 0), stop=(kc == NC_D - 1))
        nc.vector.tensor_add(pvtmp_sb[:], ps[0:AT, 0:VW], bvaug_sb[0:AT, :])
        nc.vector.tensor_mul(pv_sb[0:AT, :], pvtmp_sb[:], gbc_sb[0:AT, :])
        nc.sync.dma_start(pv_sb[AT:128, :], pv_sb[0:AT, :])

        sc0_cm.__exit__(None, None, None)
        proj_cm.__exit__(None, None, None)
        mm_cm.__exit__(None, None, None)

        # ---- attention-phase pools (reuse the projection PSUM) ----
        scp["p"] = ctx.enter_context(
            tc.tile_pool(name="scp", bufs=2, space="PSUM"))
        ctx_pool = ctx.enter_context(
            tc.tile_pool(name="ctxp", bufs=1, space="PSUM"))
        norm_pool = ctx.enter_context(tc.tile_pool(name="normp", bufs=2))
        out_pool = ctx.enter_context(tc.tile_pool(name="outp", bufs=2))
        dscr_pool = ctx.enter_context(
            tc.tile_pool(name="dscr", bufs=2, space="DRAM"))

        cps = {}       # pair -> [2 ctx psum accumulators]
        evstate = {}   # pair -> list of per-half finish state

        def ctx_mm(c, half, tci):
            h = 2 * c + half
            lhsT = v_sb[:, tci * VW + h * 65: tci * VW + h * 65 + 65]
            for sb in range(2):
                nc.tensor.matmul(
                    cps[c][half][:, sb * 512:(sb + 1) * 512], lhsT,
                    exps[c][half][:, tci * S + sb * 512:
                                  tci * S + (sb + 1) * 512],
                    start=(tci == 0), stop=(tci == NC_S - 1))

        def stage1_dens(c):
            """Main-ctx denominator rows -> SBUF (ACT; DMA cannot read
            PSUM) -> DMA-reshape across partitions. Runs right after the
            front-loaded ctx(c) matmuls finish."""
            evstate[c] = []
            for half in range(2):
                den_c = norm_pool.tile([1, S], F32, tag="den", bufs=4,
                                       name=f"den_{c}_{half}")
                nc.scalar.copy(den_c[:], cps[c][half][64:65, :])
                dresh = norm_pool.tile([128, 16], F32, tag="dresh", bufs=4,
                                       name=f"dr_{c}_{half}")
                nc.sync.dma_start(dresh[:, 0:8], den_c[:])
                evstate[c].append({"dresh": dresh, "cps": cps[c][half]})

        def stage1_prefix(c):
            """Prefix-ctx matmuls + fast psum evacuation (the prefix
            denominator rides in row 64 of the copy)."""
            for half in range(2):
                h = 2 * c + half
                hp = half * 64
                pps = scp["p"].tile([128, S], F32, tag="sc",
                                    name=f"pps_{c}_{half}")
                for sb in range(2):
                    nc.tensor.matmul(
                        pps[0:65, sb * 512:(sb + 1) * 512],
                        pv_sb[hp:hp + 64, h * 65: h * 65 + 65],
                        pexps[c][hp:hp + 64, sb * 512:(sb + 1) * 512],
                        tile_position=(hp, 0))
                pe_ev = norm_pool.tile([65, S], F32, tag="pe_ev", bufs=4,
                                       name=f"pe_{c}_{half}")
                nc.vector.tensor_copy(pe_ev[:], pps[0:65, :])
                st = evstate[c][half]
                nc.sync.dma_start(st["dresh"][:, 8:16], pe_ev[64:65, :])
                st["pe_ev"] = pe_ev

        def stage1_recip(c):
            """Reciprocal of both denominators, broadcast via DRAM."""
            for half in range(2):
                st = evstate[c][half]
                rrec = norm_pool.tile([128, 16], F32, tag="rrec", bufs=4,
                                      name=f"rr_{c}_{half}")
                nc.vector.reciprocal(rrec[:], st["dresh"][:])
                r_d = dscr_pool.tile([1, 2 * S], F32, tag="rd", bufs=4,
                                     name=f"rd_{c}_{half}")
                nc.sync.dma_start(r_d[0:1, 0:S], rrec[:, 0:8])
                nc.sync.dma_start(r_d[0:1, S:2 * S], rrec[:, 8:16])
                r_bc = norm_pool.tile([64, 2 * S], F32, tag="rbc", bufs=4,
                                      name=f"rbc_{c}_{half}")
                r_src = bass.AP(r_d[:].tensor, r_d[:].offset,
                                [[0, 64], [1, 2 * S]])
                nc.sync.dma_start(r_bc[:], r_src)
                st["r_bc"] = r_bc

        def stage1_cemul(c, half):
            """Normalize main ctx straight out of PSUM (frees the ctx
            accumulator for the next pair)."""
            h = 2 * c + half
            st = evstate[c][half]
            ce_n = norm_pool.tile([64, S], F32, tag="ce", bufs=4,
                                  name=f"ce_{h}")
            nc.vector.tensor_mul(ce_n[:], st["cps"][0:64, :],
                                 st["r_bc"][:, 0:S])
            st["ce_n"] = ce_n

        def stage2_thunks(c):
            """Prefix normalize + combine + store for pair c, spread
            through the following block on GpSimd."""
            thunks = []
            for half in range(2):
                h = 2 * c + half
                st = evstate[c][half]
                def t(h=h, st=st):
                    pe_n = out_pool.tile([64, S], F32, tag="pe", bufs=2,
                                         name=f"pen_{h}")
                    nc.gpsimd.tensor_mul(pe_n[:], st["pe_ev"][0:64, :],
                                         st["r_bc"][:, S:2 * S])
                    ot = out_pool.tile([64, S], F32, tag="ot", bufs=2,
                                       name=f"ot_{h}")
                    nc.gpsimd.tensor_add(ot[:], st["ce_n"][:], pe_n[:])
                    nc.sync.dma_start(outT[h * 64:(h + 1) * 64, :], ot[:])
                thunks.append(t)
            return thunks

        def attention_block(c):
            """Scores for pair c interleaved with the full finish of pair
            c-1: ctx matmuls front-loaded into the first half so the
            denominator chain completes (and the ctx psum frees) within
            this block."""
            p = c - 1
            cps[p] = [ctx_pool.tile([65, S], F32, tag=f"ctx{i}",
                                    name=f"cps_{p}_{i}") for i in range(2)]
            s2 = stage2_thunks(c - 2) if c >= 2 else []
            for tci in range(NC_S):
                scores_tci(c, tci, exps[c])
                if tci < 4:
                    for half in range(2):
                        ctx_mm(p, half, 2 * tci)
                        ctx_mm(p, half, 2 * tci + 1)
                if tci < len(s2):
                    s2[tci]()
                if tci == 3:
                    stage1_dens(p)
                elif tci == 4:
                    stage1_prefix(p)
                elif tci == 5:
                    stage1_recip(p)
                elif tci == 6:
                    stage1_cemul(p, 0)
                elif tci == 7:
                    stage1_cemul(p, 1)
            prefix_scores(c, pexps[c])

        for c in range(1, PAIRS):
            exps[c] = [exp_pool.tile([128, NC_S * S], BF16, tag="exp",
                                     name=f"exp_{c}_{i}") for i in range(2)]
            pexps[c] = pexp_pool.tile([128, S], BF16, tag="pexp",
                                      name=f"pexp_{c}")
            attention_block(c)

        # ---- trailing block: ctx(5) dense + finishes ----
        c = PAIRS - 1
        cps[c] = [ctx_pool.tile([65, S], F32, tag=f"ctx{i}",
                                name=f"cps_{c}_{i}") for i in range(2)]
        s2 = stage2_thunks(c - 1)
        for tci in range(4):
            for half in range(2):
                ctx_mm(c, half, 2 * tci)
                ctx_mm(c, half, 2 * tci + 1)
            if tci < len(s2):
                s2[tci]()
        stage1_dens(c)
        stage1_prefix(c)
        stage1_recip(c)
        stage1_cemul(c, 0)
        stage1_cemul(c, 1)
        for t in stage2_thunks(c):
            t()


def _prep_inputs(hidden_states, prompt_tokens, gating_factor, attention_mask,
                 Wq, bq, Wk, bk, Wv, bv):
    bf = ml_dtypes.bfloat16
    hs = np.asarray(hidden_states, np.float32)
    mask = np.asarray(attention_mask, np.float32).reshape(B, S)
    wqT = np.ascontiguousarray(np.asarray(Wq, np.float32).T).astype(bf)
    wkT = np.ascontiguousarray(np.asarray(Wk, np.float32).T).astype(bf)
    # augmented WvT: [din, 780], col 65h+j = Wv.T[:, 64h+j], col 65h+64 = 0
    wvT_f = np.asarray(Wv, np.float32).T  # [din, dout]
    wvT_aug = np.zeros((D, VW), np.float32)
    idx = np.arange(D)
    aug_cols = (idx // DH) * (DH + 1) + (idx % DH)
    wvT_aug[:, aug_cols] = wvT_f
    wvT_aug = wvT_aug.astype(bf)
    bq_c = np.asarray(bq, np.float32).reshape(D, 1)
    bk_c = np.asarray(bk, np.float32).reshape(D, 1)
    bv_aug = np.zeros(VW, np.float32)
    bv_aug[aug_cols] = np.asarray(bv, np.float32)
    bv_aug[DH::DH + 1] = 1.0
    bvaug_bc = np.ascontiguousarray(
        np.broadcast_to(bv_aug, (128, VW)), np.float32)
    pT = np.ascontiguousarray(
        np.asarray(prompt_tokens, np.float32)[0].T).astype(bf)
    gat_row = np.repeat(
        np.asarray(gating_factor, np.float32).reshape(H), DH + 1)
    gat = np.ascontiguousarray(
        np.broadcast_to(gat_row, (128, VW)), np.float32)

    shared = dict(wqT=wqT, wkT=wkT, wvT=wvT_aug, bq=bq_c, bk=bk_c,
                  bvaug=bvaug_bc, promptT=pT, gating=gat)
    in_maps = []
    for b in range(B):
        m = dict(shared)
        m["hsT"] = np.ascontiguousarray(hs[b].T).astype(bf)
        m["mask"] = np.ascontiguousarray(mask[b].reshape(S, 1))
        in_maps.append(m)
    return in_maps


def kernel(**inputs):
    global LAST_RESULTS
    if "nc" not in _CACHE:
        _CACHE["nc"] = _build_nc()
    nc = _CACHE["nc"]
    in_maps = _prep_inputs(**inputs)
    res = None
    for attempt in range(3):
        try:
            res = run_bass_kernel_spmd(nc, in_maps, list(range(B)))
            break
        except ModuleNotFoundError:
            # BASS_TRACE set but this image lacks antenv.axon_hooks
            import os

            os.environ["BASS_NEVER_TRACE"] = "1"
            if attempt == 2:
                raise
        except Exception:
            # transient NRT_EXEC_UNIT_UNRECOVERABLE on a cold device has
            # been observed; a retry on the same session recovers
            if attempt == 2:
                raise
    LAST_RESULTS = res
    out = np.empty((B, S, D), np.float32)
    for b in range(B):
        out[b] = res.results[b]["outT"].T
    return out


# revision 18
# speedup vs baseline: 1.2202x; 1.2202x over previous
"""BertSelfAttention with gated prompt-prefix branch on 8 Trainium2 cores.

Sharding: data-parallel over batch (B=8 -> 1 batch element per core), no
collectives. Per core, the full attention pipeline runs in a transposed
[feature, seq] layout so that softmax statistics ride through the matmuls:

  qT/kT = W @ hsT          [768, 1024]  (bf16, PE)
  v_aug = hs @ WvT_aug     [1024, 780]  natural layout, 65-col stride per
                           head, col 65h+64 = ones (denominator column)
  scoresT_h = kh @ qh.T    [t, s] via K=64 row-tiled matmuls, 2 heads
                           concurrently on the 128x128 PE array
  expT = exp(SCALE*scoresT)  split between the ACT engine (true exp) and
         the DVE (Schraudolph bit-trick: bf16_bits = int16(x*a + b)) so
         that neither drain engine ever paces the PE.  The PE p-state
         ramp resets whenever a PE instruction blocks on an unsatisfied
         semaphore, halving matmul throughput; the whole schedule is
         built to keep every PE wait pre-satisfied.
  ctxT_aug_h = v_aug_h.T @ expT_h       rows 0..63 ctx, row 64 = sum_t exp
  ctx matmuls for pair c-1 interleave with scores for pair c (a full
  pair of lag), and the Q/K/V/prompt projections are spread through the
  pair-0 score phase as PE filler.
  out_h = ctxT/denom + pctxT/pdenom     (DVE mul from PSUM + GpSimd
          mul/add; reciprocal + partition-broadcast ride the DMA rings)

Output is produced as outT [768, 1024] fp32 per core; the host transposes
and stacks to [8, 1024, 768].
"""

import numpy as np
import ml_dtypes

import concourse.bass as bass
import concourse.mybir as mybir
import concourse.tile as tile
from concourse.bass_utils import run_bass_kernel_spmd
from concourse.vector_clock import ScopedClock


class SplitDrainTileContext(tile.TileContext):
    """This walrus build rejects >2 sync waits on the kernel-tail Drain
    ("Too many sync wait commands"); split them across SP nops instead."""

    def _drain_and_barrier(self, tick_clock, wait_clock):
        probe = self.nc.sync.nop(nofuse=True, hint="drain_wait_split")
        wait_clock.add_sem_waits(
            probe.ins, ScopedClock({None: tick_clock.global_clock})
        )
        waits = list(probe.ins.sync_info.on_wait or [])
        if len(waits) > 1:
            probe.ins.sync_info.on_wait = waits[:1]
            for i in range(1, len(waits)):
                extra = self.nc.sync.nop(nofuse=True, hint="drain_wait_split")
                extra.ins.sync_info = mybir.SyncInfo(
                    on_wait=waits[i : i + 1], on_update=[]
                )
        drain_inst = self.nc.sync.drain()
        if drain_inst.ins.sync_info is not None:
            drain_inst.ins.sync_info.on_wait = []
        self.nc.all_engine_barrier()
        assert self.sems is not None
        popped = self.nc._tile_sem_poison_stack.pop()
        assert popped is self._sem_poison
        self.nc.clear_and_free_semaphores(list(self.sems.allocated().values()))
        self.nc.all_engine_barrier()

F32 = mybir.dt.float32
BF16 = mybir.dt.bfloat16
I16 = mybir.dt.int16
AF = mybir.ActivationFunctionType
ALU = mybir.AluOpType

H, DH, D = 12, 64, 768
S, AT, B = 1024, 64, 8
SCALE = 1.0 / np.sqrt(DH)
NC_D = D // 128  # 6 chunks over feature dim
NC_S = S // 128  # 8 chunks over sequence dim
PAIRS = H // 2  # 6 head pairs
VW = H * (DH + 1)  # 780: v with per-head ones column

# Schraudolph exp -> bf16 bits: bits = trunc(x * EXP_A + EXP_B), bitcast
# int16 -> bf16 gives ~exp(SCALE*x) with ~1.8% rms error that averages
# out inside the softmax-weighted context sum.
EXP_A = float(SCALE * 128.0 / np.log(2.0))
EXP_B = 16256.0 - 6.75

_CACHE = {}
LAST_RESULTS = None


_DVE_TILES = {(1, 1), (2, 1), (3, 0), (3, 1), (4, 1), (5, 1), (6, 1)}


def _exp_engine(tci, half):
    # DVE (Schraudolph) takes 7 of the 16 main exp tiles per pair; ACT the
    # rest + prefix. Tiles in the score-only half of the block (tci>=4) are
    # exp'd in two 512-col ops so draining starts after the first matmul —
    # the psum-rotation window there is too short for a full-tile drain.
    return ("dve" if (tci, half) in _DVE_TILES else "act",
            tci >= 4)


def _split_sync_waits(nc, cap=1):
    """Walrus on this image allows very few sync-wait commands per
    instruction (tensor_scalar rejects 2). Hoist excess waits onto
    same-engine nops placed immediately before the instruction."""
    for bb in nc.main_func.blocks:
        cur = list(bb.instructions)
        out = []
        for inst in cur:
            si = inst.sync_info
            waits = list(si.on_wait) if si and si.on_wait else []
            if len(waits) > cap:
                for i in range(0, len(waits) - cap):
                    bi = nc.engines[inst.engine].nop(
                        nofuse=True, hint="wait_split")
                    popped = nc.cur_bb.bb.instructions.pop()
                    assert popped is bi.ins
                    bi.ins.sync_info = mybir.SyncInfo(
                        on_wait=waits[i : i + 1], on_update=[])
                    out.append(bi.ins)
                si.on_wait = waits[len(waits) - cap:]
            out.append(inst)
        bb.instructions[:] = out


def _build_nc():
    nc = bass.Bass()
    hsT = nc.dram_tensor("hsT", [D, S], BF16, kind="ExternalInput")
    wqT = nc.dram_tensor("wqT", [D, D], BF16, kind="ExternalInput")
    wkT = nc.dram_tensor("wkT", [D, D], BF16, kind="ExternalInput")
    wvT = nc.dram_tensor("wvT", [D, VW], BF16, kind="ExternalInput")
    bq = nc.dram_tensor("bq", [D, 1], F32, kind="ExternalInput")
    bk = nc.dram_tensor("bk", [D, 1], F32, kind="ExternalInput")
    bvaug = nc.dram_tensor("bvaug", [128, VW], F32, kind="ExternalInput")
    promptT = nc.dram_tensor("promptT", [D, AT], BF16, kind="ExternalInput")
    mask = nc.dram_tensor("mask", [S, 1], F32, kind="ExternalInput")
    gating = nc.dram_tensor("gating", [128, VW], F32, kind="ExternalInput")
    outT = nc.dram_tensor("outT", [D, S], F32, kind="ExternalOutput")

    with SplitDrainTileContext(nc) as tc:
        _emit(nc, tc, hsT, wqT, wkT, wvT, bq, bk, bvaug, promptT, mask,
              gating, outT)
    _split_sync_waits(nc)
    return nc


def _emit(nc, tc, hsT, wqT, wkT, wvT, bq, bk, bvaug, promptT, mask, gating,
          outT):
    from contextlib import ExitStack

    with ExitStack() as ctx:
        pers = ctx.enter_context(tc.tile_pool(name="pers", bufs=1))

        # ---- SBUF arrays that live into the attention phase ----
        mask_sb = pers.tile([128, NC_S], F32, tag="mask")
        emask_sb = pers.tile([128, NC_S], F32, tag="emask")
        qT_sb = pers.tile([128, NC_D * S], BF16, tag="qT")
        kT_sb = pers.tile([128, NC_D * S], BF16, tag="kT")
        v_sb = pers.tile([128, NC_S * VW], BF16, tag="v")
        pkT_sb = pers.tile([128, NC_D * AT], BF16, tag="pkT")
        pv_sb = pers.tile([128, VW], BF16, tag="pv")

        # ---- projection-phase-only arrays (pool closed afterwards so the
        # attention pools can reuse the space) ----
        proj_cm = tc.tile_pool(name="proj", bufs=1, side="right")
        proj = proj_cm.__enter__()
        hsT_sb = proj.tile([128, NC_D * S], BF16, tag="hsT")
        wqT_sb = proj.tile([128, NC_D * D], BF16, tag="wqT")
        wkT_sb = proj.tile([128, NC_D * D], BF16, tag="wkT")
        wvT_sb = proj.tile([128, NC_D * VW], BF16, tag="wvT")
        pT_sb = proj.tile([128, NC_D * AT], BF16, tag="pT")
        bq_sb = proj.tile([128, NC_D], F32, tag="bq")
        bk_sb = proj.tile([128, NC_D], F32, tag="bk")
        bvaug_sb = proj.tile([128, VW], F32, tag="bvaug")
        graw_sb = proj.tile([128, VW], F32, tag="graw")
        gbc_sb = proj.tile([128, VW], F32, tag="gbc")
        pvtmp_sb = proj.tile([64, VW], F32, tag="pvtmp")

        for src, dst, w in ((wqT, wqT_sb, D), (hsT, hsT_sb, S),
                            (wkT, wkT_sb, D), (wvT, wvT_sb, VW),
                            (promptT, pT_sb, AT)):
            nc.sync.dma_start(
                dst[:].rearrange("p (c s) -> p c s", s=w),
                src[:, :].rearrange("(c p) s -> p c s", p=128))
        # biases / mask: [768,1] & [1024,1] -> [128, nchunks]
        nc.sync.dma_start(bq_sb[:], bq.rearrange("(c p) 1 -> p c", p=128))
        nc.sync.dma_start(bk_sb[:], bk.rearrange("(c p) 1 -> p c", p=128))
        nc.sync.dma_start(mask_sb[:], mask.rearrange("(c p) 1 -> p c", p=128))
        nc.sync.dma_start(bvaug_sb[:], bvaug[:])
        # gating arrives host-replicated to [128, 780] (65 copies per head
        # along the row, broadcast down the partitions)
        nc.sync.dma_start(graw_sb[:], gating[:])
        # tanh, then force the ones-column slots back to 1.0
        nc.scalar.activation(gbc_sb[:], graw_sb[:], AF.Tanh)
        ones_slots = gbc_sb[:, :].rearrange(
            "p (h e) -> p h e", h=H)[:, :, DH:DH + 1]
        nc.vector.memset(ones_slots, 1.0)
        # e^mask, folded into the V rows (incl. ones column) instead of an
        # exp bias: exp(S*x + m_t) == e^{m_t} * exp(S*x), and the ones
        # column then accumulates the correctly-masked denominator.
        nc.scalar.activation(emask_sb[:], mask_sb[:], AF.Exp)

        # SBUF pools that outlive the projection phase — opened before the
        # closeable PSUM pools so the per-side pool stack unwinds LIFO
        exp_pool = ctx.enter_context(tc.tile_pool(name="expp", bufs=4))
        pexp_pool = ctx.enter_context(tc.tile_pool(name="pexpp", bufs=3))

        # ---- PSUM pools for the projection phase (closed afterwards) ----
        mm_cm = tc.tile_pool(name="mm", bufs=2, space="PSUM")
        mm_pool = mm_cm.__enter__()
        sc0_cm = tc.tile_pool(name="scp0", bufs=2, space="PSUM")
        scp = {"p": sc0_cm.__enter__()}

        def emit_exp(dst, src, engine):
            if engine == "act":
                nc.scalar.activation(dst, src, AF.Exp, scale=SCALE)
            else:
                nc.vector.tensor_scalar(
                    dst.bitcast(I16), src, EXP_A, EXP_B,
                    op0=ALU.mult, op1=ALU.add)

        def scores_tci(c, tci, exp_ab):
            """Scores + exp for (pair c, key-chunk tci), 2 heads row-tiled."""
            for half in range(2):
                hp = half * 64
                st = scp["p"].tile([128, S], F32, tag="sc",
                                   name=f"st_{c}_{tci}_{half}")
                lhsT = kT_sb[hp:hp + 64,
                             c * S + tci * 128: c * S + (tci + 1) * 128]
                eng, split = _exp_engine(tci, half)
                dst = exp_ab[half][:, tci * S:(tci + 1) * S]
                for sb in range(2):
                    nc.tensor.matmul(
                        st[:, sb * 512:(sb + 1) * 512], lhsT,
                        qT_sb[hp:hp + 64,
                              c * S + sb * 512: c * S + (sb + 1) * 512],
                        tile_position=(hp, 0))
                    if split:
                        emit_exp(dst[:, sb * 512:(sb + 1) * 512],
                                 st[:, sb * 512:(sb + 1) * 512], eng)
                if not split:
                    emit_exp(dst, st[:], eng)

        psp_live = {}

        def prefix_scores_half(c, half, pexp):
            """One head's prefix scores (2 matmuls); the exp fires once the
            second half lands. Split across two emission slots so the
            score-psum rotation keeps a steady cadence."""
            if half == 0:
                psp_live[c] = scp["p"].tile([128, S], F32, tag="sc",
                                            name=f"psp_{c}")
            psp = psp_live[c]
            hp = half * 64
            for sb in range(2):
                nc.tensor.matmul(
                    psp[hp:hp + 64, sb * 512:(sb + 1) * 512],
                    pkT_sb[hp:hp + 64, c * AT:(c + 1) * AT],
                    qT_sb[hp:hp + 64,
                          c * S + sb * 512: c * S + (sb + 1) * 512],
                    tile_position=(hp, hp))
            if half == 1:
                nc.scalar.activation(pexp[:], psp[:], AF.Exp, scale=SCALE)

        def prefix_scores(c, pexp):
            prefix_scores_half(c, 0, pexp)
            prefix_scores_half(c, 1, pexp)

        # ---- projection helpers (run as PE filler between score chunks) ----
        def qk_chain(c, w_sb, b_sb, o_sb):
            ps = mm_pool.tile([128, S], F32, tag="mm")
            for kc in range(NC_D):
                lhsT = w_sb[:, kc * D + c * 128: kc * D + (c + 1) * 128]
                for sb in range(2):
                    nc.tensor.matmul(
                        ps[:, sb * 512:(sb + 1) * 512], lhsT,
                        hsT_sb[:, kc * S + sb * 512: kc * S + (sb + 1) * 512],
                        start=(kc == 0), stop=(kc == NC_D - 1))
            nc.vector.tensor_scalar_add(o_sb[:, c * S:(c + 1) * S],
                                        ps[:], b_sb[:, c:c + 1])

        def v_chunk(sc):
            ps = mm_pool.tile([128, S], F32, tag="mm")
            for kc in range(NC_D):
                lhsT = hsT_sb[:, kc * S + sc * 128: kc * S + (sc + 1) * 128]
                nc.tensor.matmul(ps[:, 0:512], lhsT,
                                 wvT_sb[:, kc * VW: kc * VW + 512],
                                 start=(kc == 0), stop=(kc == NC_D - 1))
                nc.tensor.matmul(ps[:, 512:VW], lhsT,
                                 wvT_sb[:, kc * VW + 512: (kc + 1) * VW],
                                 start=(kc == 0), stop=(kc == NC_D - 1))
            vt = proj.tile([128, VW], F32, tag="vtmp", name=f"vt{sc}",
                           bufs=2)
            nc.vector.tensor_add(vt[:], ps[:, 0:VW], bvaug_sb[:])
            nc.vector.tensor_scalar_mul(v_sb[:, sc * VW:(sc + 1) * VW],
                                        vt[:], emask_sb[:, sc:sc + 1])

        # ---- projection phase: pair-0 scores ride between filler chains ----
        qk_chain(0, wqT_sb, bq_sb, qT_sb)
        qk_chain(0, wkT_sb, bk_sb, kT_sb)
        fillers = []
        for c in range(1, NC_D):
            fillers.append(lambda c=c: qk_chain(c, wqT_sb, bq_sb, qT_sb))
            fillers.append(lambda c=c: qk_chain(c, wkT_sb, bk_sb, kT_sb))
        exps = {0: [exp_pool.tile([128, NC_S * S], BF16, tag="exp",
                                  name=f"exp_0_{i}") for i in range(2)]}
        fi = 0
        for tci in range(NC_S):
            scores_tci(0, tci, exps[0])
            n = 2 if tci < 2 else 1
            for _ in range(n):
                if fi < len(fillers):
                    fillers[fi]()
                    fi += 1
            v_chunk(tci)
        while fi < len(fillers):
            fillers[fi]()
            fi += 1

        # ---- prompt K projection (transposed) ----
        for c in range(NC_D):
            ps = mm_pool.tile([128, S], F32, tag="mm")
            for kc in range(NC_D):
                nc.tensor.matmul(
                    ps[:, 0:AT],
                    wkT_sb[:, kc * D + c * 128: kc * D + (c + 1) * 128],
                    pT_sb[:, kc * AT:(kc + 1) * AT],
                    start=(kc == 0), stop=(kc == NC_D - 1))
            nc.vector.tensor_scalar_add(pkT_sb[:, c * AT:(c + 1) * AT],
                                        ps[:, 0:AT], bk_sb[:, c:c + 1])

        # pair-0 prefix scores before the prompt-V chain so pexp(0) lands
        # well before the first attention block consumes the banks
        pexps = {0: pexp_pool.tile([128, S], BF16, tag="pexp", name="pexp0")}
        prefix_scores(0, pexps[0])

        # ---- prompt V projection (natural, gate-scaled, duplicated) ----
        ps = mm_pool.tile([128, S], F32, tag="mm")
        for kc in range(NC_D):
            lhsT = pT_sb[:, kc * AT:(kc + 1) * AT]
            nc.tensor.matmul(ps[0:AT, 0:512], lhsT,
                             wvT_sb[:, kc * VW: kc * VW + 512],
                             start=(kc == 0), stop=(kc == NC_D - 1))
            nc.tensor.matmul(ps[0:AT, 512:VW], lhsT,
                             wvT_sb[:, kc * VW + 512: (kc + 1) * VW],
                             start=(kc == 0), stop=(kc == NC_D - 1))
        nc.vector.tensor_add(pvtmp_sb[:], ps[0:AT, 0:VW], bvaug_sb[0:AT, :])
        nc.vector.tensor_mul(pv_sb[0:AT, :], pvtmp_sb[:], gbc_sb[0:AT, :])
        nc.sync.dma_start(pv_sb[AT:128, :], pv_sb[0:AT, :])

        sc0_cm.__exit__(None, None, None)
        proj_cm.__exit__(None, None, None)
        mm_cm.__exit__(None, None, None)

        # ---- attention-phase pools (reuse the projection PSUM) ----
        scp["p"] = ctx.enter_context(
            tc.tile_pool(name="scp", bufs=2, space="PSUM"))
        ctx_pool = ctx.enter_context(
            tc.tile_pool(name="ctxp", bufs=1, space="PSUM"))
        norm_pool = ctx.enter_context(tc.tile_pool(name="normp", bufs=2))
        out_pool = ctx.enter_context(tc.tile_pool(name="outp", bufs=2))
        dscr_pool = ctx.enter_context(
            tc.tile_pool(name="dscr", bufs=2, space="DRAM"))

        cps = {}       # pair -> [2 ctx psum accumulators]
        evstate = {}   # pair -> list of per-half finish state

        def ctx_mm(c, half, tci):
            h = 2 * c + half
            lhsT = v_sb[:, tci * VW + h * 65: tci * VW + h * 65 + 65]
            for sb in range(2):
                nc.tensor.matmul(
                    cps[c][half][:, sb * 512:(sb + 1) * 512], lhsT,
                    exps[c][half][:, tci * S + sb * 512:
                                  tci * S + (sb + 1) * 512],
                    start=(tci == 0), stop=(tci == NC_S - 1))

        def stage1_dens(c, half):
            """Main-ctx denominator row -> SBUF (ACT; DMA cannot read
            PSUM) -> DMA-reshape across partitions. Runs right after the
            front-loaded ctx(c, half) matmuls finish."""
            if half == 0:
                evstate[c] = []
            den_c = norm_pool.tile([1, S], F32, tag="den", bufs=4,
                                   name=f"den_{c}_{half}")
            nc.scalar.copy(den_c[:], cps[c][half][64:65, :])
            dresh = norm_pool.tile([128, 16], F32, tag="dresh", bufs=4,
                                   name=f"dr_{c}_{half}")
            nc.sync.dma_start(dresh[:, 0:8], den_c[:])
            evstate[c].append({"dresh": dresh, "cps": cps[c][half]})

        def stage1_prefix(c, half):
            """Prefix-ctx matmuls + fast psum evacuation (the prefix
            denominator rides in row 64 of the copy)."""
            h = 2 * c + half
            hp = half * 64
            pps = scp["p"].tile([128, S], F32, tag="sc",
                                name=f"pps_{c}_{half}")
            for sb in range(2):
                nc.tensor.matmul(
                    pps[0:65, sb * 512:(sb + 1) * 512],
                    pv_sb[hp:hp + 64, h * 65: h * 65 + 65],
                    pexps[c][hp:hp + 64, sb * 512:(sb + 1) * 512],
                    tile_position=(hp, 0))
            pe_ev = norm_pool.tile([65, S], F32, tag="pe_ev", bufs=4,
                                   name=f"pe_{c}_{half}")
            nc.vector.tensor_copy(pe_ev[:], pps[0:65, :])
            st = evstate[c][half]
            nc.sync.dma_start(st["dresh"][:, 8:16], pe_ev[64:65, :])
            st["pe_ev"] = pe_ev

        def stage1_recip(c, half):
            """Reciprocal of one head's denominators, broadcast via DRAM."""
            st = evstate[c][half]
            rrec = norm_pool.tile([128, 16], F32, tag="rrec", bufs=4,
                                  name=f"rr_{c}_{half}")
            nc.vector.reciprocal(rrec[:], st["dresh"][:])
            r_d = dscr_pool.tile([1, 2 * S], F32, tag="rd", bufs=4,
                                 name=f"rd_{c}_{half}")
            nc.sync.dma_start(r_d[0:1, 0:S], rrec[:, 0:8])
            nc.sync.dma_start(r_d[0:1, S:2 * S], rrec[:, 8:16])
            r_bc = norm_pool.tile([64, 2 * S], F32, tag="rbc", bufs=4,
                                  name=f"rbc_{c}_{half}")
            r_src = bass.AP(r_d[:].tensor, r_d[:].offset,
                            [[0, 64], [1, 2 * S]])
            nc.sync.dma_start(r_bc[:], r_src)
            st["r_bc"] = r_bc

        def stage1_cemul(c, half):
            """Normalize main ctx straight out of PSUM (frees the ctx
            accumulator for the next pair)."""
            h = 2 * c + half
            st = evstate[c][half]
            ce_n = norm_pool.tile([64, S], F32, tag="ce", bufs=4,
                                  name=f"ce_{h}")
            nc.vector.tensor_mul(ce_n[:], st["cps"][0:64, :],
                                 st["r_bc"][:, 0:S])
            st["ce_n"] = ce_n

        def stage2_thunks(c):
            """Prefix normalize + combine + store for pair c, spread
            through the following block on GpSimd."""
            thunks = []
            for half in range(2):
                h = 2 * c + half
                st = evstate[c][half]
                def t(h=h, st=st):
                    pe_n = out_pool.tile([64, S], F32, tag="pe", bufs=2,
                                         name=f"pen_{h}")
                    nc.gpsimd.tensor_mul(pe_n[:], st["pe_ev"][0:64, :],
                                         st["r_bc"][:, S:2 * S])
                    ot = out_pool.tile([64, S], F32, tag="ot", bufs=2,
                                       name=f"ot_{h}")
                    nc.gpsimd.tensor_add(ot[:], st["ce_n"][:], pe_n[:])
                    nc.sync.dma_start(outT[h * 64:(h + 1) * 64, :], ot[:])
                thunks.append(t)
            return thunks

        def attention_block(c):
            """Scores for pair c interleaved with the full finish of pair
            c-1. ctx matmuls are front-loaded (h0 over tci 0-3, h1 over
            tci 1-4, staggered so each half's psum bank is reused only
            after the previous pair's normalization read it), and the
            prefix/denominator work spreads through the score-only tail so
            the PE keeps a steady cadence there."""
            p = c - 1
            cps[p] = [ctx_pool.tile([65, S], F32, tag=f"ctx{i}",
                                    name=f"cps_{p}_{i}") for i in range(2)]
            s2 = stage2_thunks(c - 2) if c >= 2 else []
            for tci in range(NC_S):
                scores_tci(c, tci, exps[c])
                if tci < 4:
                    ctx_mm(p, 0, 2 * tci)
                    ctx_mm(p, 0, 2 * tci + 1)
                if 1 <= tci < 5:
                    ctx_mm(p, 1, 2 * (tci - 1))
                    ctx_mm(p, 1, 2 * (tci - 1) + 1)
                if 1 <= tci < 1 + len(s2):
                    s2[tci - 1]()
                if tci == 3:
                    stage1_dens(p, 0)
                elif tci == 4:
                    stage1_dens(p, 1)
                    stage1_prefix(p, 0)
                elif tci == 5:
                    stage1_prefix(p, 1)
                    stage1_recip(p, 0)
                elif tci == 6:
                    stage1_recip(p, 1)
                    prefix_scores_half(c, 0, pexps[c])
                    stage1_cemul(p, 0)
                elif tci == 7:
                    prefix_scores_half(c, 1, pexps[c])
                    stage1_cemul(p, 1)

        for c in range(1, PAIRS):
            exps[c] = [exp_pool.tile([128, NC_S * S], BF16, tag="exp",
                                     name=f"exp_{c}_{i}") for i in range(2)]
            pexps[c] = pexp_pool.tile([128, S], BF16, tag="pexp",
                                      name=f"pexp_{c}")
            attention_block(c)

        # ---- trailing block: ctx(5) dense + finishes ----
        c = PAIRS - 1
        cps[c] = [ctx_pool.tile([65, S], F32, tag=f"ctx{i}",
                                name=f"cps_{c}_{i}") for i in range(2)]
        s2 = stage2_thunks(c - 1)
        for tci in range(4):
            ctx_mm(c, 0, 2 * tci)
            ctx_mm(c, 0, 2 * tci + 1)
            ctx_mm(c, 1, 2 * tci)
            ctx_mm(c, 1, 2 * tci + 1)
            if tci < len(s2):
                s2[tci]()
        stage1_dens(c, 0)
        stage1_dens(c, 1)
        stage1_prefix(c, 0)
        stage1_prefix(c, 1)
        stage1_recip(c, 0)
        stage1_recip(c, 1)
        stage1_cemul(c, 0)
        stage1_cemul(c, 1)
        for t in stage2_thunks(c):
            t()


def _prep_inputs(hidden_states, prompt_tokens, gating_factor, attention_mask,
                 Wq, bq, Wk, bk, Wv, bv):
    bf = ml_dtypes.bfloat16
    hs = np.asarray(hidden_states, np.float32)
    mask = np.asarray(attention_mask, np.float32).reshape(B, S)
    wqT = np.ascontiguousarray(np.asarray(Wq, np.float32).T).astype(bf)
    wkT = np.ascontiguousarray(np.asarray(Wk, np.float32).T).astype(bf)
    # augmented WvT: [din, 780], col 65h+j = Wv.T[:, 64h+j], col 65h+64 = 0
    wvT_f = np.asarray(Wv, np.float32).T  # [din, dout]
    wvT_aug = np.zeros((D, VW), np.float32)
    idx = np.arange(D)
    aug_cols = (idx // DH) * (DH + 1) + (idx % DH)
    wvT_aug[:, aug_cols] = wvT_f
    wvT_aug = wvT_aug.astype(bf)
    bq_c = np.asarray(bq, np.float32).reshape(D, 1)
    bk_c = np.asarray(bk, np.float32).reshape(D, 1)
    bv_aug = np.zeros(VW, np.float32)
    bv_aug[aug_cols] = np.asarray(bv, np.float32)
    bv_aug[DH::DH + 1] = 1.0
    bvaug_bc = np.ascontiguousarray(
        np.broadcast_to(bv_aug, (128, VW)), np.float32)
    pT = np.ascontiguousarray(
        np.asarray(prompt_tokens, np.float32)[0].T).astype(bf)
    gat_row = np.repeat(
        np.asarray(gating_factor, np.float32).reshape(H), DH + 1)
    gat = np.ascontiguousarray(
        np.broadcast_to(gat_row, (128, VW)), np.float32)

    shared = dict(wqT=wqT, wkT=wkT, wvT=wvT_aug, bq=bq_c, bk=bk_c,
                  bvaug=bvaug_bc, promptT=pT, gating=gat)
    in_maps = []
    for b in range(B):
        m = dict(shared)
        m["hsT"] = np.ascontiguousarray(hs[b].T).astype(bf)
        m["mask"] = np.ascontiguousarray(mask[b].reshape(S, 1))
        in_maps.append(m)
    return in_maps


def kernel(**inputs):
    global LAST_RESULTS
    if "nc" not in _CACHE:
        _CACHE["nc"] = _build_nc()
    nc = _CACHE["nc"]
    in_maps = _prep_inputs(**inputs)
    res = None
    for attempt in range(3):
        try:
            res = run_bass_kernel_spmd(nc, in_maps, list(range(B)))
            break
        except ModuleNotFoundError:
            # BASS_TRACE set but this image lacks antenv.axon_hooks
            import os

            os.environ["BASS_NEVER_TRACE"] = "1"
            if attempt == 2:
                raise
        except Exception:
            # transient NRT_EXEC_UNIT_UNRECOVERABLE on a cold device has
            # been observed; a retry on the same session recovers
            if attempt == 2:
                raise
    LAST_RESULTS = res
    out = np.empty((B, S, D), np.float32)
    for b in range(B):
        out[b] = res.results[b]["outT"].T
    return out
